# revision 1
# baseline (speedup 1.0000x reference)
"""Trainium2 Bass kernel for CafeEmbeddingBagCollection (moe_routing).

Reference op: for each of N=204800 flat tokens, route to one of two
embedding tables (hot table if query_result < 0, else hash table at
q % HASH), then sum-pool the per-token rows into B=4096 bags given by
`offsets`, producing [B, 128] f32.

Strategy (data-parallel, tables replicated on all 8 cores):
  * Host does LAYOUT ONLY: concatenates [hot_W; hash_W; zero_row] into a
    single [1000002, 128] table, and rearranges each core's query_results
    into a bag-per-partition layout [128, n_chunks * Lmax] (sentinel-padded
    when bags are uneven) so that partition p of chunk k holds the tokens
    of bag (k*128 + p).
  * Device computes the hot/hash routing arithmetic per chunk (including
    q % HASH via an exact f32 reciprocal-multiply with +-1 correction),
    gathers rows with one indirect DMA per token column (HW consumes one
    offset per partition per call), pools each bag with a single reduce_sum
    over the token axis, and writes [128, 128] per chunk to the output.
  * Host concatenates the 8 per-core [512, 128] outputs.
"""

import os
import sys

import numpy as np

sys.path.insert(0, "/opt/trn_rl_repo")

# Problem constants (hardcoded per harness contract).
B = 4096
L = 50
N = B * L
D = 128
HOT = 500000
HASH = 500000
NCORES = 8
BC = B // NCORES  # bags per core = 512
CHUNKS = BC // 128  # 128-bag chunks per core = 4

TROWS = HOT + 1 + HASH + 1  # 1000002: [hot | hash | zero row]
ZR = TROWS - 1  # index of the all-zero row (padding target)
PADVAL = 1 << 30  # sentinel query value for padded token slots

_CACHE: dict = {}


def _build_nc(lmax: int, tsub: int):
    """Build the SPMD Bass program for bags padded to lmax tokens,
    gathered in sub-slices of tsub tokens per bag."""
    import concourse.bacc as bacc
    import concourse.bass as bass
    import concourse.tile as tile
    from concourse import mybir

    M = CHUNKS * lmax  # routed-index columns per partition

    nc = bacc.Bacc(
        "TRN2",
        target_bir_lowering=False,
        debug=False,
        num_devices=NCORES,
    )

    q_in = nc.dram_tensor("q", [128, M], mybir.dt.int32, kind="ExternalInput")
    table_in = nc.dram_tensor(
        "table", [TROWS, D], mybir.dt.float32, kind="ExternalInput"
    )
    out_dram = nc.dram_tensor(
        "out", [BC, D], mybir.dt.float32, kind="ExternalOutput"
    )

    f32 = mybir.dt.float32
    i32 = mybir.dt.int32
    Alu = mybir.AluOpType

    with tile.TileContext(nc) as tc:
        with (
            tc.tile_pool(name="route", bufs=2) as route,
            tc.tile_pool(name="gath", bufs=3) as gath,
            tc.tile_pool(name="accp", bufs=2) as accp,
        ):
            # per-chunk routing: chunk 0's gathers start ~3.5us sooner
            def route_cols(c0, w):
                q = route.tile([128, w], i32, tag="q")
                nc.sync.dma_start(out=q[:], in_=q_in[:, c0 : c0 + w])
                qf = route.tile([128, w], f32, tag="qf")
                nc.vector.tensor_copy(qf[:], q[:])
                hot = route.tile([128, w], f32, tag="hot")
                nc.vector.tensor_scalar(hot[:], qf[:], -1.0, float(HOT), op0=Alu.mult, op1=Alu.min)
                kf = route.tile([128, w], f32, tag="kf")
                nc.vector.tensor_scalar_mul(kf[:], qf[:], 1.0 / HASH)
                ki = route.tile([128, w], i32, tag="ki")
                nc.vector.tensor_copy(ki[:], kf[:])
                nc.vector.tensor_copy(kf[:], ki[:])
                r = route.tile([128, w], f32, tag="r")
                nc.vector.tensor_scalar_mul(kf[:], kf[:], float(HASH))
                nc.vector.tensor_tensor(r[:], qf[:], kf[:], op=Alu.subtract)
                c1 = route.tile([128, w], f32, tag="c1")
                nc.vector.tensor_scalar(c1[:], r[:], 0.0, float(HASH), op0=Alu.is_lt, op1=Alu.mult)
                nc.vector.tensor_tensor(r[:], r[:], c1[:], op=Alu.add)
                nc.vector.tensor_scalar(c1[:], r[:], float(HASH), float(HASH), op0=Alu.is_ge, op1=Alu.mult)
                nc.vector.tensor_tensor(r[:], r[:], c1[:], op=Alu.subtract)
                nc.vector.tensor_scalar_add(r[:], r[:], float(HOT + 1))
                idxf = route.tile([128, w], f32, tag="idxf")
                mask = route.tile([128, w], i32, tag="mask")
                nc.vector.tensor_scalar(mask[:], qf[:], 0.0, None, op0=Alu.is_lt)
                nc.vector.select(idxf[:], mask[:], hot[:], r[:])
                pmask = route.tile([128, w], i32, tag="pmask")
                nc.vector.tensor_scalar(pmask[:], qf[:], float(PADVAL), None, op0=Alu.is_equal)
                zr = route.tile([128, w], f32, tag="zr")
                nc.vector.memset(zr[:], float(ZR))
                nc.vector.copy_predicated(idxf[:], pmask[:], zr[:])
                idx_c = route.tile([128, w], i32, tag="idx_c")
                nc.vector.tensor_copy(idx_c[:], idxf[:])
                return idx_c

            # ---- gather + pool per 128-bag chunk ----
            for ch in range(CHUNKS):
                idx = route_cols(ch * lmax, lmax)
                acc = accp.tile([128, D], f32)
                nsub = (lmax + tsub - 1) // tsub
                for si in range(nsub):
                    t0 = si * tsub
                    ts = min(tsub, lmax - t0)
                    g = gath.tile([128, tsub * D], f32, tag="g")
                    # HW DGE consumes exactly one offset per partition per
                    # indirect DMA (scalar-per-partition mode): issue one call
                    # per token column, each gathering 128 rows.
                    for j in range(ts):
                        nc.gpsimd.indirect_dma_start(
                            out=g[:, j * D : (j + 1) * D],
                            out_offset=None,
                            in_=table_in[:],
                            in_offset=bass.IndirectOffsetOnAxis(
                                ap=idx[:, t0 + j : t0 + j + 1],
                                axis=0,
                            ),
                            bounds_check=TROWS - 1,
                            oob_is_err=False,
                        )
                    # view [p, d, t] (d stride 1, token stride D) -> reduce tokens
                    g3 = g[:, : ts * D].rearrange("p (t d) -> p d t", d=D)
                    if si == 0:
                        nc.vector.reduce_sum(acc[:], g3, axis=mybir.AxisListType.X)
                    else:
                        part = accp.tile([128, D], f32, tag="part")
                        nc.vector.reduce_sum(part[:], g3, axis=mybir.AxisListType.X)
                        nc.vector.tensor_tensor(acc[:], acc[:], part[:], op=Alu.add)
                nc.sync.dma_start(
                    out=out_dram[ch * 128 : (ch + 1) * 128, :], in_=acc[:]
                )

    nc.compile()
    return nc


def _arrange_tokens(query_results: np.ndarray, offsets: np.ndarray):
    """Bag-per-partition token layout. Returns (arranged [B, lmax] int32, lmax)."""
    starts = offsets.astype(np.int64)
    ends = np.empty_like(starts)
    ends[:-1] = starts[1:]
    ends[-1] = N
    lens = np.maximum(ends - starts, 0)
    lmax = int(lens.max()) if lens.size else 0
    uniform = bool((starts == np.arange(B, dtype=np.int64) * L).all())
    if uniform:
        return query_results.reshape(B, L).astype(np.int32), L
    arranged = np.full((B, lmax), PADVAL, dtype=np.int32)
    for b in range(B):
        s, e = starts[b], ends[b]
        if e > s:
            arranged[b, : e - s] = query_results[s:e]
    return arranged, lmax


def kernel(feature_ids, offsets, query_results, hot_W, hash_W):
    from concourse.bass_utils import run_bass_kernel_spmd

    query_results = np.asarray(query_results, dtype=np.int32)
    offsets = np.asarray(offsets, dtype=np.int32)
    hot_W = np.ascontiguousarray(np.asarray(hot_W, dtype=np.float32))
    hash_W = np.ascontiguousarray(np.asarray(hash_W, dtype=np.float32))

    table = np.empty((TROWS, D), dtype=np.float32)
    table[: HOT + 1] = hot_W
    table[HOT + 1 : HOT + 1 + HASH] = hash_W
    table[ZR] = 0.0

    arranged, lmax = _arrange_tokens(query_results, offsets)
    # tokens-per-bag slice size per gather: keep gather tiles ~<=3.3MB
    tsub = min(lmax, 50) if lmax else 1
    lmax = max(lmax, 1)

    key = (lmax, tsub)
    if key not in _CACHE:
        _CACHE[key] = _build_nc(lmax, tsub)
    nc = _CACHE[key]

    in_maps = []
    for c in range(NCORES):
        rows = arranged[c * BC : (c + 1) * BC]  # [512, lmax]
        q_arr = (
            rows.reshape(CHUNKS, 128, lmax)
            .transpose(1, 0, 2)
            .reshape(128, CHUNKS * lmax)
        )
        in_maps.append({"q": np.ascontiguousarray(q_arr), "table": table})

    r = run_bass_kernel_spmd(nc, in_maps, list(range(NCORES)))
    globals()["LAST_RESULTS"] = r  # exposes exec_time_ns/trace to test harness
    out = np.concatenate([r.results[c]["out"] for c in range(NCORES)], axis=0)
    return out.astype(np.float32)



# revision 4
# speedup vs baseline: 1.0051x; 1.0051x over previous
"""Trainium2 Bass kernel for CafeEmbeddingBagCollection (moe_routing).

Reference op: for each of N=204800 flat tokens, route to one of two
embedding tables (hot table at |q| if query_result q < 0, else hash table
at q % HASH), then sum-pool the per-token rows into B=4096 bags given by
`offsets`, producing [B, 128] f32.

Strategy (data-parallel, tables replicated on all 8 cores):
  * Host LAYOUT: one combined [1000001, 128] table laid out as
    [reversed hot_W[1:] | hash_W | zero row].  With the hot rows reversed,
    the device route for token q becomes branch-free arithmetic:
        row = min(q mod HASH, q) + HASH
    (hot ids are negative: min picks q, and HASH+q indexes the reversed hot
    row for |q|; hash ids are >= 0: q mod HASH <= q so min picks the mod.)
    query_results are rearranged host-side into a bag-per-partition layout
    [128, CHUNKS*lmax] so partition p of chunk k holds bag (k*128+p).
  * Device: routing is 2 DVE ops per 128-bag chunk (scalar_tensor_tensor
    fusing mod+min, then +HASH).  Each token-slice of a chunk is gathered
    with a SINGLE indirect DMA carrying one offset per token (the SWDGE
    consumes the whole [128, ts] offset AP in one instruction, amortizing
    the ~1us per-instruction descriptor-generation overhead over up to 6400
    rows).  Pooling is a DVE reduce over the token axis; chunk 0 and the
    last chunk are token-sliced so the first transfer starts early (fill)
    and the last reduce is small (drain); everything else overlaps the
    DMA-engine gather time, which is the memory roofline for this op.
  * Host concatenates the 8 per-core [512, 128] outputs.
"""

import os
import sys

import numpy as np

sys.path.insert(0, "/opt/trn_rl_repo")

# Problem constants (hardcoded per harness contract).
B = 4096
L = 50
N = B * L
D = 128
HOT = 500000
HASH = 500000
NCORES = 8
BC = B // NCORES  # bags per core = 512
CHUNKS = BC // 128  # 128-bag chunks per core = 4

TROWS = HOT + HASH + 1  # 1000001: [reversed hot | hash | zero row]
ZR = TROWS - 1  # index of the all-zero row (padding target)
PADVAL = 1 << 30  # sentinel query value for padded token slots

_CACHE: dict = {}


def _slice_plan(lmax: int) -> list[list[int]]:
    """Token-slice widths per chunk. Small first slice (prompt the DMA
    engines early) and small trailing slices (short exposed tail reduce)."""
    if lmax >= 24:
        first = [12, lmax - 12]
        mid = [lmax]
        tail_a = max(lmax - 20, 1)
        rest = lmax - tail_a
        tail = [tail_a] + ([rest - rest // 2, rest // 2] if rest >= 2 else ([rest] if rest else []))
    else:
        first = [lmax]
        mid = [lmax]
        tail = [lmax]
    plans = []
    for ch in range(CHUNKS):
        if ch == 0:
            plans.append(first)
        elif ch == CHUNKS - 1:
            plans.append([t for t in tail if t > 0])
        else:
            plans.append(mid)
    return plans


def _build_nc(lmax: int, uniform: bool):
    """SPMD Bass program for bags padded to lmax tokens."""
    import concourse.bacc as bacc
    import concourse.bass as bass
    import concourse.tile as tile
    from concourse import mybir

    M = CHUNKS * lmax

    nc = bacc.Bacc(
        "TRN2",
        target_bir_lowering=False,
        debug=False,
        num_devices=NCORES,
    )

    q_in = nc.dram_tensor("q", [128, M], mybir.dt.int32, kind="ExternalInput")
    table_in = nc.dram_tensor(
        "table", [TROWS, D], mybir.dt.float32, kind="ExternalInput"
    )
    out_dram = nc.dram_tensor(
        "out", [BC, D], mybir.dt.float32, kind="ExternalOutput"
    )

    f32 = mybir.dt.float32
    i32 = mybir.dt.int32
    Alu = mybir.AluOpType

    plans = _slice_plan(lmax)

    with tile.TileContext(nc) as tc:
        with (
            tc.tile_pool(name="io", bufs=1) as io,
            tc.tile_pool(name="route", bufs=2) as route,
            tc.tile_pool(name="gath", bufs=3) as gath,
            tc.tile_pool(name="accp", bufs=2) as accp,
        ):
            qall = io.tile([128, M], i32, tag="qall")
            nc.sync.dma_start(out=qall[:], in_=q_in[:])

            if not uniform:
                # holds ZR - HASH: the +HASH in the route lands padded
                # slots exactly on the zero row ZR.
                zrow = io.tile([128, lmax], f32, tag="zrow")
                nc.vector.memset(zrow[:], float(ZR - HASH))

            # Routing: 2 DVE ops per chunk (plus sentinel fixup when padded).
            idxs = []
            for ch in range(CHUNKS):
                q_sl = qall[:, ch * lmax : (ch + 1) * lmax]
                rmin = route.tile([128, lmax], f32, tag="rmin")
                # rmin = min(q mod HASH, q)
                nc.vector.scalar_tensor_tensor(
                    rmin[:], q_sl, float(HASH), q_sl, op0=Alu.mod, op1=Alu.min
                )
                if not uniform:
                    pmask = route.tile([128, lmax], i32, tag="pmask")
                    nc.vector.tensor_scalar(
                        pmask[:], q_sl, PADVAL, None, op0=Alu.is_equal
                    )
                    nc.vector.copy_predicated(rmin[:], pmask[:], zrow[:])
                idx = route.tile([128, lmax], i32, tag=f"idx{ch}")
                nc.vector.tensor_scalar_add(idx[:], rmin[:], float(HASH))
                idxs.append(idx)

            # gather + pool, token-sliced per chunk
            for ch in range(CHUNKS):
                idx = idxs[ch]
                acc = accp.tile([128, D], f32, tag="acc")
                t0 = 0
                for si, ts in enumerate(plans[ch]):
                    g = gath.tile([128, lmax * D], f32, tag="g")
                    nc.gpsimd.indirect_dma_start(
                        out=g[:, : ts * D],
                        out_offset=None,
                        in_=table_in[:],
                        in_offset=bass.IndirectOffsetOnAxis(
                            ap=idx[:, t0 : t0 + ts],
                            axis=0,
                        ),
                        bounds_check=TROWS - 1,
                        oob_is_err=False,
                    )
                    g3 = g[:, : ts * D].rearrange("p (t d) -> p d t", d=D)
                    if si == 0:
                        nc.vector.reduce_sum(acc[:], g3, axis=mybir.AxisListType.X)
                    else:
                        part = accp.tile([128, D], f32, tag="part")
                        nc.vector.reduce_sum(part[:], g3, axis=mybir.AxisListType.X)
                        nc.vector.tensor_tensor(acc[:], acc[:], part[:], op=Alu.add)
                    t0 += ts
                nc.sync.dma_start(
                    out=out_dram[ch * 128 : (ch + 1) * 128, :], in_=acc[:]
                )

    nc.compile()
    return nc


def _arrange_tokens(query_results: np.ndarray, offsets: np.ndarray):
    """Bag-per-partition token layout. Returns (arranged [B, lmax] int32,
    lmax, uniform)."""
    starts = offsets.astype(np.int64)
    ends = np.empty_like(starts)
    ends[:-1] = starts[1:]
    ends[-1] = N
    lens = np.maximum(ends - starts, 0)
    lmax = int(lens.max()) if lens.size else 0
    uniform = bool((starts == np.arange(B, dtype=np.int64) * L).all())
    if uniform:
        return query_results.reshape(B, L).astype(np.int32), L, True
    arranged = np.full((B, lmax), PADVAL, dtype=np.int32)
    for b in range(B):
        s, e = starts[b], ends[b]
        if e > s:
            arranged[b, : e - s] = query_results[s:e]
    return arranged, lmax, False


def _build_table(hot_W: np.ndarray, hash_W: np.ndarray) -> np.ndarray:
    table = np.empty((TROWS, D), dtype=np.float32)
    # reversed hot rows: row (HASH + q) for q in [-HOT, -1] holds hot_W[-q]
    table[:HOT] = hot_W[1 : HOT + 1][::-1]
    table[HOT : HOT + HASH] = hash_W
    table[ZR] = 0.0
    return table


def kernel(feature_ids, offsets, query_results, hot_W, hash_W):
    from concourse.bass_utils import run_bass_kernel_spmd

    query_results = np.asarray(query_results, dtype=np.int32)
    offsets = np.asarray(offsets, dtype=np.int32)
    hot_W = np.ascontiguousarray(np.asarray(hot_W, dtype=np.float32))
    hash_W = np.ascontiguousarray(np.asarray(hash_W, dtype=np.float32))

    table = _build_table(hot_W, hash_W)

    arranged, lmax, uniform = _arrange_tokens(query_results, offsets)
    lmax = max(lmax, 1)

    key = (lmax, uniform)
    if key not in _CACHE:
        _CACHE[key] = _build_nc(lmax, uniform)
    nc = _CACHE[key]

    in_maps = []
    for c in range(NCORES):
        rows = arranged[c * BC : (c + 1) * BC]  # [512, lmax]
        q_arr = (
            rows.reshape(CHUNKS, 128, lmax)
            .transpose(1, 0, 2)
            .reshape(128, CHUNKS * lmax)
        )
        in_maps.append({"q": np.ascontiguousarray(q_arr), "table": table})

    r = run_bass_kernel_spmd(nc, in_maps, list(range(NCORES)))
    globals()["LAST_RESULTS"] = r  # exposes exec_time_ns/trace to test harness
    out = np.concatenate([r.results[c]["out"] for c in range(NCORES)], axis=0)
    return out.astype(np.float32)


# revision 34
# speedup vs baseline: 1.0304x; 1.0251x over previous
"""Trainium2 Bass kernel for CafeEmbeddingBagCollection (moe_routing).

Reference op: for each of N=204800 flat tokens, route to one of two
embedding tables (hot table at |q| if query_result q < 0, else hash table
at q % HASH), then sum-pool the per-token rows into B=4096 bags given by
`offsets`, producing [B, 128] f32.

Strategy (data-parallel, tables replicated on all 8 cores):
  * Host does LAYOUT ONLY: concatenates [hot_W; hash_W; zero_row] into a
    single [1000002, 128] table, and rearranges each core's query_results
    into a bag-per-partition layout [128, CHUNKS * lmax] (sentinel-padded
    when bags are uneven) so that partition p of chunk k holds the tokens
    of bag (k*128 + p).
  * Device computes the hot/hash routing per chunk (q % HASH via an exact
    f32 reciprocal-multiply with +-1 correction), then gathers rows with
    one indirect DMA per token column (the DGE consumes one offset per
    partition per call).  Pooling rides the DMA itself: every column of a
    chunk lands on the same [128, D] accumulator with compute_op=add
    (column 0 writes), so no vector-engine reduction is needed at all and
    the sum-pool is finished the moment the last column lands.  Columns
    are issued round-robin across the four 128-bag chunks so each
    accumulator's write->accumulate chain never stalls the descriptor
    generator.
  * Host concatenates the 8 per-core [512, 128] outputs.
"""

import os
import sys

import numpy as np

sys.path.insert(0, "/opt/trn_rl_repo")

# Problem constants (hardcoded per harness contract).
B = 4096
L = 50
N = B * L
D = 128
HOT = 500000
HASH = 500000
NCORES = 8
BC = B // NCORES  # bags per core = 512
CHUNKS = BC // 128  # 128-bag chunks per core = 4

TROWS = HOT + 1 + HASH + 1  # 1000002: [hot | hash | zero row]
ZR = TROWS - 1  # index of the all-zero row (padding target)
PADVAL = 1 << 30  # sentinel query value for padded token slots

_CACHE: dict = {}


def _build_nc(lmax: int):
    """Build the SPMD Bass program for bags padded to lmax tokens."""
    import concourse.bacc as bacc
    import concourse.bass as bass
    import concourse.tile as tile
    from concourse import mybir

    M = CHUNKS * lmax

    nc = bacc.Bacc(
        "TRN2",
        target_bir_lowering=False,
        debug=False,
        num_devices=NCORES,
    )

    q_in = nc.dram_tensor("q", [128, M], mybir.dt.int32, kind="ExternalInput")
    table_in = nc.dram_tensor(
        "table", [TROWS, D], mybir.dt.float32, kind="ExternalInput"
    )
    out_dram = nc.dram_tensor(
        "out", [BC, D], mybir.dt.float32, kind="ExternalOutput"
    )

    f32 = mybir.dt.float32
    i32 = mybir.dt.int32
    Alu = mybir.AluOpType

    with tile.TileContext(nc) as tc:
        with (
            tc.tile_pool(name="route", bufs=2) as route,
            tc.tile_pool(name="accp", bufs=1) as accp,
        ):
            # routing math per chunk (baseline-proven sequence)
            def route_cols(c0, w):
                q = route.tile([128, w], i32, tag="q")
                nc.sync.dma_start(out=q[:], in_=q_in[:, c0 : c0 + w])
                qf = route.tile([128, w], f32, tag="qf")
                nc.vector.tensor_copy(qf[:], q[:])
                hot = route.tile([128, w], f32, tag="hot")
                nc.vector.tensor_scalar(hot[:], qf[:], -1.0, float(HOT), op0=Alu.mult, op1=Alu.min)
                kf = route.tile([128, w], f32, tag="kf")
                nc.vector.tensor_scalar_mul(kf[:], qf[:], 1.0 / HASH)
                ki = route.tile([128, w], i32, tag="ki")
                nc.vector.tensor_copy(ki[:], kf[:])
                nc.vector.tensor_copy(kf[:], ki[:])
                r = route.tile([128, w], f32, tag="r")
                nc.vector.tensor_scalar_mul(kf[:], kf[:], float(HASH))
                nc.vector.tensor_tensor(r[:], qf[:], kf[:], op=Alu.subtract)
                c1 = route.tile([128, w], f32, tag="c1")
                nc.vector.tensor_scalar(c1[:], r[:], 0.0, float(HASH), op0=Alu.is_lt, op1=Alu.mult)
                nc.vector.tensor_tensor(r[:], r[:], c1[:], op=Alu.add)
                nc.vector.tensor_scalar(c1[:], r[:], float(HASH), float(HASH), op0=Alu.is_ge, op1=Alu.mult)
                nc.vector.tensor_tensor(r[:], r[:], c1[:], op=Alu.subtract)
                nc.vector.tensor_scalar_add(r[:], r[:], float(HOT + 1))
                idxf = route.tile([128, w], f32, tag="idxf")
                mask = route.tile([128, w], i32, tag="mask")
                nc.vector.tensor_scalar(mask[:], qf[:], 0.0, None, op0=Alu.is_lt)
                nc.vector.select(idxf[:], mask[:], hot[:], r[:])
                pmask = route.tile([128, w], i32, tag="pmask")
                nc.vector.tensor_scalar(pmask[:], qf[:], float(PADVAL), None, op0=Alu.is_equal)
                zr = route.tile([128, w], f32, tag="zr")
                nc.vector.memset(zr[:], float(ZR))
                nc.vector.copy_predicated(idxf[:], pmask[:], zr[:])
                idx_c = route.tile([128, w], i32, tag=f"idx_c{c0}", name="idx_c")
                nc.vector.tensor_copy(idx_c[:], idxf[:])
                return idx_c

            idxs = [route_cols(ch * lmax, lmax) for ch in range(CHUNKS)]
            accs = [
                accp.tile([128, D], f32, tag=f"acc{ch}", name=f"acc{ch}")
                for ch in range(CHUNKS)
            ]

            # Sum-pool on the DMA: column j of chunk ch gathers 128 rows
            # (one offset per partition) straight onto acc[ch] with
            # compute_op=add.  Round-robin over chunks gives each
            # accumulator chain ~4 descriptor slots (~4us) between its
            # consecutive columns, far more than the transfer+semaphore
            # latency, so the pool engine never stalls.
            for j in range(lmax):
                for ch in range(CHUNKS):
                    nc.gpsimd.indirect_dma_start(
                        out=accs[ch][:],
                        out_offset=None,
                        in_=table_in[:],
                        in_offset=bass.IndirectOffsetOnAxis(
                            ap=idxs[ch][:, j : j + 1],
                            axis=0,
                        ),
                        bounds_check=TROWS - 1,
                        oob_is_err=False,
                        compute_op=Alu.bypass if j == 0 else Alu.add,
                    )
                    if j == lmax - 1:
                        nc.sync.dma_start(
                            out=out_dram[ch * 128 : (ch + 1) * 128, :],
                            in_=accs[ch][:],
                        )

    nc.compile()
    return nc


def _arrange_tokens(query_results: np.ndarray, offsets: np.ndarray):
    """Bag-per-partition token layout. Returns (arranged [B, lmax] int32, lmax)."""
    starts = offsets.astype(np.int64)
    ends = np.empty_like(starts)
    ends[:-1] = starts[1:]
    ends[-1] = N
    lens = np.maximum(ends - starts, 0)
    lmax = int(lens.max()) if lens.size else 0
    uniform = bool((starts == np.arange(B, dtype=np.int64) * L).all())
    if uniform:
        return query_results.reshape(B, L).astype(np.int32), L
    arranged = np.full((B, lmax), PADVAL, dtype=np.int32)
    for b in range(B):
        s, e = starts[b], ends[b]
        if e > s:
            arranged[b, : e - s] = query_results[s:e]
    return arranged, lmax


def _build_table(hot_W: np.ndarray, hash_W: np.ndarray) -> np.ndarray:
    table = np.empty((TROWS, D), dtype=np.float32)
    table[: HOT + 1] = hot_W
    table[HOT + 1 : HOT + 1 + HASH] = hash_W
    table[ZR] = 0.0
    return table


def kernel(feature_ids, offsets, query_results, hot_W, hash_W):
    from concourse.bass_utils import run_bass_kernel_spmd

    query_results = np.asarray(query_results, dtype=np.int32)
    offsets = np.asarray(offsets, dtype=np.int32)
    hot_W = np.ascontiguousarray(np.asarray(hot_W, dtype=np.float32))
    hash_W = np.ascontiguousarray(np.asarray(hash_W, dtype=np.float32))

    table = _build_table(hot_W, hash_W)

    arranged, lmax = _arrange_tokens(query_results, offsets)
    lmax = max(lmax, 1)

    if lmax not in _CACHE:
        _CACHE[lmax] = _build_nc(lmax)
    nc = _CACHE[lmax]

    in_maps = []
    for c in range(NCORES):
        rows = arranged[c * BC : (c + 1) * BC]  # [512, lmax]
        q_arr = (
            rows.reshape(CHUNKS, 128, lmax)
            .transpose(1, 0, 2)
            .reshape(128, CHUNKS * lmax)
        )
        in_maps.append({"q": np.ascontiguousarray(q_arr), "table": table})

    r = run_bass_kernel_spmd(nc, in_maps, list(range(NCORES)))
    globals()["LAST_RESULTS"] = r  # exposes exec_time_ns/trace to test harness
    out = np.concatenate([r.results[c]["out"] for c in range(NCORES)], axis=0)
    return out.astype(np.float32)


# revision 37
# speedup vs baseline: 1.0307x; 1.0003x over previous
"""Trainium2 Bass kernel for CafeEmbeddingBagCollection (moe_routing).

Reference op: for each of N=204800 flat tokens, route to one of two
embedding tables (hot table at |q| if query_result q < 0, else hash table
at q % HASH), then sum-pool the per-token rows into B=4096 bags given by
`offsets`, producing [B, 128] f32.

Strategy (data-parallel, tables replicated on all 8 cores):
  * Host does LAYOUT ONLY: concatenates [hot_W; hash_W; zero_row] into a
    single [1000002, 128] table, and rearranges each core's query_results
    into a bag-per-partition layout [128, CHUNKS * lmax] (sentinel-padded
    when bags are uneven) so that partition p of chunk k holds the tokens
    of bag (k*128 + p).
  * Device computes the hot/hash routing per chunk (q % HASH via an exact
    f32 reciprocal-multiply with +-1 correction), then gathers rows with
    one indirect DMA per token column (the DGE consumes one offset per
    partition per call).  Pooling rides the DMA itself: every column of a
    chunk lands on the same [128, D] accumulator with compute_op=add
    (column 0 writes), so no vector-engine reduction is needed at all and
    the sum-pool is finished the moment the last column lands.  Columns
    are issued round-robin across the four 128-bag chunks so each
    accumulator's write->accumulate chain never stalls the descriptor
    generator.
  * Host concatenates the 8 per-core [512, 128] outputs.
"""

import os
import sys

import numpy as np

sys.path.insert(0, "/opt/trn_rl_repo")

# Problem constants (hardcoded per harness contract).
B = 4096
L = 50
N = B * L
D = 128
HOT = 500000
HASH = 500000
NCORES = 8
BC = B // NCORES  # bags per core = 512
CHUNKS = BC // 128  # 128-bag chunks per core = 4

TROWS = HOT + 1 + HASH + 1  # 1000002: [hot | hash | zero row]
ZR = TROWS - 1  # index of the all-zero row (padding target)
PADVAL = 1 << 30  # sentinel query value for padded token slots

_CACHE: dict = {}


def _build_nc(lmax: int):
    """Build the SPMD Bass program for bags padded to lmax tokens."""
    import concourse.bacc as bacc
    import concourse.bass as bass
    import concourse.tile as tile
    from concourse import mybir

    M = CHUNKS * lmax

    nc = bacc.Bacc(
        "TRN2",
        target_bir_lowering=False,
        debug=False,
        num_devices=NCORES,
    )

    q_in = nc.dram_tensor("q", [128, M], mybir.dt.int32, kind="ExternalInput")
    table_in = nc.dram_tensor(
        "table", [TROWS, D], mybir.dt.float32, kind="ExternalInput"
    )
    out_dram = nc.dram_tensor(
        "out", [BC, D], mybir.dt.float32, kind="ExternalOutput"
    )

    f32 = mybir.dt.float32
    i32 = mybir.dt.int32
    Alu = mybir.AluOpType

    with tile.TileContext(nc) as tc:
        with (
            tc.tile_pool(name="route", bufs=2) as route,
            tc.tile_pool(name="accp", bufs=1) as accp,
        ):
            # routing math per chunk (baseline-proven sequence)
            def route_cols(c0, w):
                q = route.tile([128, w], i32, tag=f"q{w}")
                nc.sync.dma_start(out=q[:], in_=q_in[:, c0 : c0 + w])
                qf = route.tile([128, w], f32, tag=f"qf{w}")
                nc.vector.tensor_copy(qf[:], q[:])
                hot = route.tile([128, w], f32, tag=f"hot{w}")
                nc.vector.tensor_scalar(hot[:], qf[:], -1.0, float(HOT), op0=Alu.mult, op1=Alu.min)
                kf = route.tile([128, w], f32, tag=f"kf{w}")
                nc.vector.tensor_scalar_mul(kf[:], qf[:], 1.0 / HASH)
                ki = route.tile([128, w], i32, tag=f"ki{w}")
                nc.vector.tensor_copy(ki[:], kf[:])
                nc.vector.tensor_copy(kf[:], ki[:])
                r = route.tile([128, w], f32, tag=f"r{w}")
                nc.vector.tensor_scalar_mul(kf[:], kf[:], float(HASH))
                nc.vector.tensor_tensor(r[:], qf[:], kf[:], op=Alu.subtract)
                c1 = route.tile([128, w], f32, tag=f"c1{w}")
                nc.vector.tensor_scalar(c1[:], r[:], 0.0, float(HASH), op0=Alu.is_lt, op1=Alu.mult)
                nc.vector.tensor_tensor(r[:], r[:], c1[:], op=Alu.add)
                nc.vector.tensor_scalar(c1[:], r[:], float(HASH), float(HASH), op0=Alu.is_ge, op1=Alu.mult)
                nc.vector.tensor_tensor(r[:], r[:], c1[:], op=Alu.subtract)
                nc.vector.tensor_scalar_add(r[:], r[:], float(HOT + 1))
                idxf = route.tile([128, w], f32, tag=f"idxf{w}")
                mask = route.tile([128, w], i32, tag=f"mask{w}")
                nc.vector.tensor_scalar(mask[:], qf[:], 0.0, None, op0=Alu.is_lt)
                nc.vector.select(idxf[:], mask[:], hot[:], r[:])
                pmask = route.tile([128, w], i32, tag=f"pmask{w}")
                nc.vector.tensor_scalar(pmask[:], qf[:], float(PADVAL), None, op0=Alu.is_equal)
                zr = route.tile([128, w], f32, tag=f"zr{w}")
                nc.vector.memset(zr[:], float(ZR))
                nc.vector.copy_predicated(idxf[:], pmask[:], zr[:])
                idx_c = route.tile([128, w], i32, tag=f"idx_c{c0}", name="idx_c")
                nc.vector.tensor_copy(idx_c[:], idxf[:])
                return idx_c

            # routes[ch] = [(lo, hi, idx_tile), ...]; chunk 0's first column
            # gets a dedicated 1-wide route chain so the very first
            # descriptor generation starts as early as possible (fill).
            routes = [[] for _ in range(CHUNKS)]
            with tc.high_priority():
                routes[0].append((0, 1, route_cols(0, 1)))
            if lmax > 1:
                routes[0].append((1, lmax, route_cols(1, lmax - 1)))
            for ch in range(1, CHUNKS):
                routes[ch].append(
                    (0, lmax, route_cols(ch * lmax, lmax))
                )

            def idx_col(ch, j):
                for lo, hi, tile_ in routes[ch]:
                    if lo <= j < hi:
                        return tile_[:, j - lo : j - lo + 1]
                raise AssertionError((ch, j))

            accs = [
                accp.tile([128, D], f32, tag=f"acc{ch}", name=f"acc{ch}")
                for ch in range(CHUNKS)
            ]

            # Sum-pool on the DMA: column j of chunk ch gathers 128 rows
            # (one offset per partition) straight onto acc[ch] with
            # compute_op=add.  Round-robin over chunks gives each
            # accumulator chain ~4 descriptor slots (~4us) between its
            # consecutive columns, far more than the transfer+semaphore
            # latency, so the pool engine never stalls.
            for j in range(lmax):
                for ch in range(CHUNKS):
                    nc.gpsimd.indirect_dma_start(
                        out=accs[ch][:],
                        out_offset=None,
                        in_=table_in[:],
                        in_offset=bass.IndirectOffsetOnAxis(
                            ap=idx_col(ch, j),
                            axis=0,
                        ),
                        bounds_check=TROWS - 1,
                        oob_is_err=False,
                        compute_op=Alu.bypass if j == 0 else Alu.add,
                    )
                    if j == lmax - 1:
                        nc.sync.dma_start(
                            out=out_dram[ch * 128 : (ch + 1) * 128, :],
                            in_=accs[ch][:],
                        )

    nc.compile()
    return nc


def _arrange_tokens(query_results: np.ndarray, offsets: np.ndarray):
    """Bag-per-partition token layout. Returns (arranged [B, lmax] int32, lmax)."""
    starts = offsets.astype(np.int64)
    ends = np.empty_like(starts)
    ends[:-1] = starts[1:]
    ends[-1] = N
    lens = np.maximum(ends - starts, 0)
    lmax = int(lens.max()) if lens.size else 0
    uniform = bool((starts == np.arange(B, dtype=np.int64) * L).all())
    if uniform:
        return query_results.reshape(B, L).astype(np.int32), L
    arranged = np.full((B, lmax), PADVAL, dtype=np.int32)
    for b in range(B):
        s, e = starts[b], ends[b]
        if e > s:
            arranged[b, : e - s] = query_results[s:e]
    return arranged, lmax


def _build_table(hot_W: np.ndarray, hash_W: np.ndarray) -> np.ndarray:
    table = np.empty((TROWS, D), dtype=np.float32)
    table[: HOT + 1] = hot_W
    table[HOT + 1 : HOT + 1 + HASH] = hash_W
    table[ZR] = 0.0
    return table


def kernel(feature_ids, offsets, query_results, hot_W, hash_W):
    from concourse.bass_utils import run_bass_kernel_spmd

    query_results = np.asarray(query_results, dtype=np.int32)
    offsets = np.asarray(offsets, dtype=np.int32)
    hot_W = np.ascontiguousarray(np.asarray(hot_W, dtype=np.float32))
    hash_W = np.ascontiguousarray(np.asarray(hash_W, dtype=np.float32))

    table = _build_table(hot_W, hash_W)

    arranged, lmax = _arrange_tokens(query_results, offsets)
    lmax = max(lmax, 1)

    if lmax not in _CACHE:
        _CACHE[lmax] = _build_nc(lmax)
    nc = _CACHE[lmax]

    in_maps = []
    for c in range(NCORES):
        rows = arranged[c * BC : (c + 1) * BC]  # [512, lmax]
        q_arr = (
            rows.reshape(CHUNKS, 128, lmax)
            .transpose(1, 0, 2)
            .reshape(128, CHUNKS * lmax)
        )
        in_maps.append({"q": np.ascontiguousarray(q_arr), "table": table})

    r = run_bass_kernel_spmd(nc, in_maps, list(range(NCORES)))
    globals()["LAST_RESULTS"] = r  # exposes exec_time_ns/trace to test harness
    out = np.concatenate([r.results[c]["out"] for c in range(NCORES)], axis=0)
    return out.astype(np.float32)
